# revision 28
# baseline (speedup 1.0000x reference)
"""Trainium2 Bass kernel for nn_DenseGraphConvNodeToEdge.

out[b,i,j,o] = y_cols[b,j,o] + y_rows[b,i,o] + y_sum[b,o] + bias[o]
  with y_cols = x @ W0.T, y_rows = x @ W1.T, y_sum = x.sum(1) @ W2.T

Strategy: output is [4,1024,1024,64] = pure memory-regime broadcast
materialization. Shard the row dim i across 8 cores (128 rows/core).

Precision/bandwidth trades (correctness gate is rel_err < 2e-2 of max):
  * output is written as bf16 -> 64 MiB/core of HBM writes.
  * main GEMM runs in fp8e4 (e4m3) with the DoubleRow perf mode, which
    streams 2 contraction pairs per column per cycle: 512-col matmul in
    ~213ns vs 427ns for bf16 (PE issue clock is 1.2GHz effective in
    this LNC config; bf16/fp32r all stream 1 col/cycle).
  * contraction layout: 33 pairs (66 rows). Pairs 0..31 = x columns
    (x[.,2k], x[.,2k+1]) against (W1[o,2k], W1[o,2k+1]) replicated per
    j; pair 32 = (ones, ones) against (base_hi, base_lo) — the base
    term base[b,j,o] = y_cols + y_sum + bias exceeds fp8 precision
    (|base| up to ~60, e4m3 mantissa err ~|v|/16), so it is carried as
    an exact-ish hi+lo fp8 pair: hi = fp8(base), lo = fp8(base - hi),
    total representation error <= ~0.13 vs an output scale of ~61.
  * base is precomputed per (b, jblk) by a small bf16 GEMM (K=65 with
    a ones row carrying y_sum + bias, computed in exact f32), split
    hi/lo on DVE, and flattened into the pair-32 slots of the rotating
    rhs buffers via SBUF->SBUF DMAs (gpsimd + vector rings).

rhs buffer free layout per 512-col chunk c (16 chunks per jblk):
  [c*1024 + i*512 + f] = pair element i of column f — "two-major" per
  chunk, which walrus' DoubleRow AP check requires ([K,2,F] APs).

Per jblk (128 j): 16 DoubleRow matmuls [128 i, 512] -> 8 PSUM tiles
[128,1024] f32, copied f32->bf16 to SBUF staging (DVE/ACT alternating),
one 2 MiB output DMA per jblk alternating sync/scalar HWDGE rings (the
last two jblks drain as 8 x 256 KiB chunks to overlap the tail).
"""

import numpy as np

B, N, C = 4, 1024, 64
N_CORES = 8
R = N // N_CORES  # 128 rows per core
K2 = C // 2 + 1   # 33 contraction pairs

_CACHE = {}


def _build():
    import concourse.tile as tile
    from concourse import bacc, mybir

    f32 = mybir.dt.float32
    bf16 = mybir.dt.bfloat16
    fp8 = mybir.dt.float8e4
    DR = mybir.MatmulPerfMode.DoubleRow

    nc = bacc.Bacc("TRN2", target_bir_lowering=False, debug=False,
                   num_devices=N_CORES)

    xt1b = nc.dram_tensor("xt1b", [C + 1, B * N], bf16, kind="ExternalInput").ap()
    xr8 = nc.dram_tensor("xr8", [K2, B * 256], fp8, kind="ExternalInput").ap()
    w1w = nc.dram_tensor("w1w", [K2, 16384], fp8, kind="ExternalInput").ap()
    w0tb = nc.dram_tensor("w0tb", [C, C], bf16, kind="ExternalInput").ap()
    w2t = nc.dram_tensor("w2t", [C, C], f32, kind="ExternalInput").ap()
    bias_row = nc.dram_tensor("bias_row", [1, C], f32, kind="ExternalInput").ap()
    out_s = nc.dram_tensor("out_s", [B, R, N, C], bf16, kind="ExternalOutput").ap()

    with tile.TileContext(nc) as tc:
        with (
            tc.tile_pool(name="const", bufs=1) as const_pool,
            tc.tile_pool(name="rhs", bufs=1) as rhs_pool,
            tc.tile_pool(name="base", bufs=16) as base_pool,
            tc.tile_pool(name="stage", bufs=3) as stage_pool,
            tc.tile_pool(name="psm", bufs=3, space="PSUM") as psum_main,
            tc.tile_pool(name="pss", bufs=2, space="PSUM") as psum_small,
        ):
            # ---- persistent SBUF state ----
            xt1_bf = const_pool.tile([C + 1, B * N], bf16, tag="xt1b")
            lhsT_sb = const_pool.tile([K2, B * 256], fp8, tag="lhsT")
            rhs_base = const_pool.tile([C + 1, C], bf16, tag="rhsb")
            w2t_sb = const_pool.tile([C, C], f32, tag="w2t")
            bias_sb = const_pool.tile([1, C], f32, tag="bias")
            xsum_sb = const_pool.tile([C, 1], f32, tag="xsum")
            rhs_bufs = [rhs_pool.tile([K2, 16384], fp8, tag=f"rhs{k}",
                                      name=f"rhs{k}")
                        for k in range(3)]

            # ---- input DMAs, spread across the DGE rings ----
            nc.sync.dma_start(rhs_bufs[0][:, :], w1w[:, :])
            nc.sync.dma_start(xt1_bf[:, N:B * N], xt1b[:, N:B * N])
            nc.scalar.dma_start(xt1_bf[:, 0:N], xt1b[:, 0:N])
            nc.scalar.dma_start(rhs_base[0:C, :], w0tb[:, :])
            nc.scalar.dma_start(rhs_bufs[1][:, :], w1w[:, :])
            nc.scalar.dma_start(rhs_bufs[2][:, :], w1w[:, :])
            nc.gpsimd.dma_start(w2t_sb[:], w2t[:, :])
            nc.gpsimd.dma_start(bias_sb[:], bias_row[:, :])
            nc.gpsimd.dma_start(lhsT_sb[:], xr8[:, :])

            copy_idx = 0  # alternate DVE / ACT for PSUM->SBUF copies
            for b in range(B):
                # xsum[c] = sum_j x[b,j,c] (bf16 in, f32 accumulate)
                nc.vector.reduce_sum(
                    xsum_sb[:], xt1_bf[0:C, b * N:(b + 1) * N],
                    axis=mybir.AxisListType.X)
                # s2_row[o] = sum_c xsum[c] * W2[o,c] + bias[o] (exact fp32)
                ps_s2 = psum_small.tile([1, C], f32, tag="pss")
                nc.tensor.matmul(ps_s2[:], xsum_sb[:], w2t_sb[:],
                                 start=True, stop=True)
                nc.vector.tensor_add(rhs_base[C:C + 1, :], ps_s2[:], bias_sb[:])

                # precompute all 8 base hi/lo tile pairs for this b up front
                base_tiles = []
                for jblk in range(8):
                    # base tile [128 j, 64 o] (bf16 GEMM, f32 accumulate)
                    ps_b = psum_small.tile([128, C], f32, tag="pss")
                    nc.tensor.matmul(
                        ps_b[:],
                        xt1_bf[:, b * N + jblk * 128: b * N + (jblk + 1) * 128],
                        rhs_base[:],
                        start=True, stop=True)
                    b_hi = base_pool.tile([128, C], fp8, tag="base",
                                          name=f"base_hi_{b}_{jblk}")
                    b_lo = base_pool.tile([128, C], fp8, tag="base",
                                          name=f"base_lo_{b}_{jblk}")
                    nc.vector.tensor_copy(b_hi[:], ps_b[:])
                    nc.vector.tensor_sub(b_lo[:], ps_b[:], b_hi[:])
                    base_tiles.append((b_hi, b_lo))

                lhsT = lhsT_sb[:, b * 256:(b + 1) * 256].rearrange(
                    "k (two m) -> k two m", two=2)
                for jblk in range(8):
                    # flatten hi/lo [128 j, 64 o] -> pair-32 slots of the rhs
                    # buffer (SWDGE gpsimd + vector rings: don't queue behind
                    # 2 MiB output DMAs on the sync/scalar HWDGE FIFOs)
                    b_hi, b_lo = base_tiles[jblk]
                    rhs = rhs_bufs[(b * 8 + jblk) % 3]
                    row32 = rhs[C // 2:C // 2 + 1, :]
                    nc.gpsimd.dma_start(
                        row32[:, 0:8192].rearrange("a (p o) -> a p o", p=128),
                        b_hi[:])
                    nc.gpsimd.dma_start(
                        row32[:, 8192:16384].rearrange("a (p o) -> a p o", p=128),
                        b_lo[:])

                    # main GEMMs: 16 x [128, 512] fp8 DoubleRow matmuls
                    last = (b == B - 1 and jblk >= 6)
                    stage_t = stage_pool.tile([128, 8192], bf16, tag="stage")
                    j0 = jblk * 128
                    r2 = rhs[:, :].rearrange("k (two g) -> k two g", two=2)
                    for g in range(8):  # psum groups of [128, 1024]
                        ps_m = psum_main.tile([128, 1024], f32, tag="psm")
                        for h in range(2):
                            c = g * 2 + h
                            nc.tensor.matmul(
                                ps_m[:, h * 512:(h + 1) * 512],
                                lhsT,
                                r2[:, :, c * 512:(c + 1) * 512],
                                start=True, stop=True, perf_mode=DR)
                        dst = stage_t[:, g * 1024:(g + 1) * 1024]
                        if copy_idx % 2 == 0:
                            nc.vector.tensor_copy(dst, ps_m[:])
                        else:
                            nc.scalar.copy(dst, ps_m[:])
                        copy_idx += 1
                        if last:
                            dma_eng = nc.sync if g % 2 == 0 else nc.scalar
                            dma_eng.dma_start(
                                out_s[b, :, j0 + g * 16:j0 + (g + 1) * 16, :],
                                dst)
                    if not last:
                        dma_eng = nc.sync if (b * 8 + jblk) % 2 == 0 else nc.scalar
                        dma_eng.dma_start(out_s[b, :, j0:j0 + 128, :], stage_t[:])

    nc.compile()
    return nc


def _get_nc():
    if "nc" not in _CACHE:
        _CACHE["nc"] = _build()
    return _CACHE["nc"]


def kernel(x, adj, W0, W1, W2, bias):
    import ml_dtypes
    from concourse.bass_utils import run_bass_kernel_spmd

    bf = ml_dtypes.bfloat16
    f8 = ml_dtypes.float8_e4m3
    x = np.ascontiguousarray(np.asarray(x, dtype=np.float32))
    W0 = np.asarray(W0, dtype=np.float32)
    W1 = np.asarray(W1, dtype=np.float32)
    W2 = np.asarray(W2, dtype=np.float32)
    bias = np.asarray(bias, dtype=np.float32)

    nc = _get_nc()

    ones_n = np.ones((B, 1, N), dtype=np.float32)
    # [C+1, B*N] bf16: partition-major with all batches in the free dim
    xt1b = np.ascontiguousarray(np.concatenate(
        [x.transpose(0, 2, 1), ones_n], axis=1).transpose(1, 0, 2)
        .reshape(C + 1, B * N).astype(bf))
    # fp8 DoubleRow rhs, plane-major pair layout: [0:8192] = pair element 0
    # (W1[o,2k] replicated per j; base_hi slots in row 32), [8192:16384] =
    # pair element 1 (W1[o,2k+1]; base_lo slots). Row 32 zero-initialized,
    # overwritten on device per jblk.
    w1rep8 = np.tile(W1.T, (1, 128)).astype(f8)        # [64, 8192]
    w1w = np.zeros((K2, 16384), dtype=f8)
    w1w[0:32, 0:8192] = w1rep8[0::2, :]
    w1w[0:32, 8192:16384] = w1rep8[1::2, :]
    w1w = np.ascontiguousarray(w1w)
    w0tb = np.ascontiguousarray(W0.T.astype(bf))
    w2t = np.ascontiguousarray(W2.T)
    bias_row = np.ascontiguousarray(bias.T)

    in_maps = []
    for core in range(N_CORES):
        xr = x[:, core * R:(core + 1) * R, :]           # [B, 128, 64]
        t = xr.transpose(2, 0, 1)                       # [64(c), B, 128]
        t = t.reshape(32, 2, B, R).transpose(0, 2, 1, 3)  # [32, B, 2, 128]
        xr8 = np.concatenate(
            [t.reshape(32, B * 256), np.ones((1, B * 256), dtype=np.float32)],
            axis=0).astype(f8)
        in_maps.append({
            "xt1b": xt1b, "xr8": np.ascontiguousarray(xr8), "w1w": w1w,
            "w0tb": w0tb, "w2t": w2t, "bias_row": bias_row,
        })

    global _last_in_maps
    _last_in_maps = in_maps
    res = run_bass_kernel_spmd(nc, in_maps, list(range(N_CORES)))

    out = np.empty((B, N, N, C), dtype=np.float32)
    for core in range(N_CORES):
        out[:, core * R:(core + 1) * R] = np.asarray(
            res.results[core]["out_s"]).astype(np.float32)
    return out
